# revision 33
# baseline (speedup 1.0000x reference)
"""Trainium2 Bass kernel for nn_MultiHeadInfoQuantizer.

Encoder: Linear(512->2048, no bias) -> LayerNorm -> ReLU -> Linear(2048->256)
Per-head (4x64) log_softmax, KL-nearest codebook lookup (1024 codes),
straight-through quantized output, masked commitment loss.

Sharding: data-parallel over N = B*T = 16384 rows; 8 cores x 2048 rows.
Weights/codebook replicated. Scalar loss partial-summed per core and
combined on host.

Returns (z, q_st, loss) matching the reference's structure.
"""

import numpy as np

import concourse.bass as bass
import concourse.tile as tile
from concourse import bacc, mybir
import concourse.bass_utils as bass_utils
from concourse.masks import make_identity

F32 = mybir.dt.float32
F32R = mybir.dt.float32r

B, T = 16, 1024
IN_CH, CH, D, M = 512, 2048, 256, 1024
NHEAD, HD = 4, 64
N_CORES = 8
ROWS_PER_CORE = B * T // N_CORES  # 2048
LN_EPS = 1e-5
COMMIT = 0.25

# Matmul dtype config: "f32" (exact, 4 cyc/row) or "f32r" (~1.6e-4 rel, 1 cyc/row)
DT_H = "f32"
DT_Z = "f32"
DT_DIV = "f32"
# Split-path h matmul: x/W1 decomposed into 11-bit-mantissa hi+lo parts and
# contracted with 3 fp32r matmuls (exact for <=11-bit inputs) -> fp32-quality
# at 3 cyc/row instead of 4.
SPLIT_H = True
SPLIT_DIV = True


def _trunc11(x):
    return (x.view(np.uint32) & np.uint32(0xFFFFF000)).view(np.float32)


def _split_hi_lo(x):
    hi = _trunc11(x)
    lo = _trunc11((x - hi).astype(np.float32))
    return np.ascontiguousarray(hi), np.ascontiguousarray(lo)

AX = mybir.AxisListType
AF = mybir.ActivationFunctionType
OP = mybir.AluOpType

# ---------------------------------------------------------------------------
# Pin the ACT piecewise-poly table choice to the one set that contains every
# function this kernel uses (Exp, Ln, Relu, Copy, Square).  The stock
# insert_act_table_loads pass picks per-function greedily and thrashes
# between exp_and_others / natural_log (~2.7us per switch, twice per tile).
# We only alter which sets the *chooser* believes contain these functions;
# set ids / runtime table contents are untouched.
_COMBINED_SET = "natural_log_exp_and_others"
_orig_get_tables = None


def _patched_get_tables(arch):
    import concourse.hw_specs as hw_specs
    tabs = _orig_get_tables(arch)
    pinned = {AF.Exp, AF.Ln, AF.Relu, AF.Copy, AF.Square, AF.Identity}
    out = {}
    for name, fns in tabs.items():
        if name == _COMBINED_SET:
            out[name] = set(fns)
        else:
            out[name] = set(fns) - pinned
    return out


def _install_table_patch():
    global _orig_get_tables
    if _orig_get_tables is None:
        import concourse.hw_specs as hw_specs
        _orig_get_tables = hw_specs.get_activation_tables
        bacc.get_activation_tables = _patched_get_tables


def _dt(name):
    return F32R if name == "f32r" else F32


def build(ntiles=ROWS_PER_CORE // 128, dt_h=None, dt_z=None, dt_div=None,
          trivial_ln=True, trivial_b2=True, stages=3, split_h=False,
          split_div=False):
    """Build the per-core SPMD program. Each core handles ntiles*128 rows.

    stages: 1 = through z output only; 2 = + softmax/P/argmax/loss; 3 = full.
    """
    _install_table_patch()
    dt_h = _dt(dt_h or DT_H)
    dt_z = _dt(dt_z or DT_Z)
    dt_div = _dt(dt_div or DT_DIV)
    rows = ntiles * 128

    nc = bacc.Bacc("TRN2", target_bir_lowering=False, debug=False,
                   num_devices=N_CORES)

    # ---- DRAM tensors ----
    if split_h:
        xs = nc.dram_tensor("xs", [IN_CH, rows], F32R, kind="ExternalInput").ap()
        xl = nc.dram_tensor("xl", [IN_CH, rows], F32R, kind="ExternalInput").ap()
        w1 = nc.dram_tensor("w1", [IN_CH, CH], F32R, kind="ExternalInput").ap()
        w1l = nc.dram_tensor("w1l", [IN_CH, CH], F32R, kind="ExternalInput").ap()
        dt_h = F32R
    else:
        xs = nc.dram_tensor("xs", [rows, IN_CH], F32, kind="ExternalInput").ap()
        w1 = nc.dram_tensor("w1", [IN_CH, CH], dt_h, kind="ExternalInput").ap()
    mks = nc.dram_tensor("mks", [rows, 1], F32, kind="ExternalInput").ap()
    w2 = nc.dram_tensor("w2", [CH, D], dt_z, kind="ExternalInput").ap()
    if split_div:
        dt_div = F32R
    dct = nc.dram_tensor("dct", [D, M], dt_div, kind="ExternalInput").ap()
    if split_div:
        dctl = nc.dram_tensor("dctl", [D, M], F32R, kind="ExternalInput").ap()
    cvec = nc.dram_tensor("cvec", [1, D], F32, kind="ExternalInput").ap()
    emb = nc.dram_tensor("emb", [M, D], F32, kind="ExternalInput").ap()
    if not trivial_ln:
        gd = nc.dram_tensor("gd", [1, CH], F32, kind="ExternalInput").ap()
        bd = nc.dram_tensor("bd", [1, CH], F32, kind="ExternalInput").ap()
    if not trivial_b2:
        b2d = nc.dram_tensor("b2d", [1, D], F32, kind="ExternalInput").ap()

    z_out = nc.dram_tensor("z_out", [rows, D], F32, kind="ExternalOutput").ap()
    q_out = nc.dram_tensor("q_out", [rows, D], F32, kind="ExternalOutput").ap()
    lsum = nc.dram_tensor("lsum", [1, 1], F32, kind="ExternalOutput").ap()
    idx_scr = nc.dram_tensor("idx_scr", [rows, 1], mybir.dt.int16,
                             kind="Internal").ap()

    with tile.TileContext(nc) as tc:
        with (
            tc.tile_pool(name="singles", bufs=1) as singles,
            tc.tile_pool(name="io", bufs=3) as io,
            tc.tile_pool(name="work", bufs=2) as work,
            tc.tile_pool(name="ph", bufs=2, space="PSUM") as ph,
            tc.tile_pool(name="ptx", bufs=2, space="PSUM") as ptx,
            tc.tile_pool(name="pth", bufs=2, space="PSUM") as pth,
            tc.tile_pool(name="ppq", bufs=1, space="PSUM") as ppq,
            tc.tile_pool(name="pz", bufs=1, space="PSUM") as pz,
        ):
            # ---- residents ----
            ident = singles.tile([128, 128], F32)
            make_identity(nc, ident)
            if split_h or dt_h is F32R:
                ident_r = singles.tile([128, 128], F32R)
                nc.vector.tensor_copy(ident_r, ident)
            eps_t = singles.tile([128, 1], F32)
            nc.vector.memset(eps_t, LN_EPS)
            ones_t = singles.tile([128, 1], F32)
            nc.vector.memset(ones_t, 1.0)
            klacc = singles.tile([128, 1], F32)
            nc.vector.memset(klacc, 0.0)

            w1sb = singles.tile([128, IN_CH // 128, CH], dt_h)
            w1r = w1.rearrange("(kc p) n -> p kc n", p=128)
            nc.sync.dma_start(out=w1sb[:, 0, :], in_=w1r[:, 0, :])
            for kc in range(1, IN_CH // 128):
                nc.scalar.dma_start(out=w1sb[:, kc, :], in_=w1r[:, kc, :])
            w2sb = singles.tile([128, CH // 128, D], dt_z)
            if split_h:
                w1lsb = singles.tile([128, IN_CH // 128, CH], F32R)
                w1lr = w1l.rearrange("(kc p) n -> p kc n", p=128)
                for kc in range(IN_CH // 128):
                    nc.scalar.dma_start(out=w1lsb[:, kc, :], in_=w1lr[:, kc, :])
            nc.scalar.dma_start(out=w2sb, in_=w2.rearrange("(kc p) n -> p kc n", p=128))
            dcsb = singles.tile([128, D // 128, M], dt_div)
            nc.scalar.dma_start(out=dcsb, in_=dct.rearrange("(kc p) m -> p kc m", p=128))
            if split_div:
                dclsb = singles.tile([128, D // 128, M], F32R)
                nc.scalar.dma_start(out=dclsb,
                                    in_=dctl.rearrange("(kc p) m -> p kc m", p=128))
            cb = singles.tile([128, D], F32)
            nc.gpsimd.dma_start(out=cb, in_=cvec.to_broadcast((128, D)))
            if not trivial_ln:
                gbt = singles.tile([128, CH], F32)
                nc.scalar.dma_start(out=gbt, in_=gd.to_broadcast((128, CH)))
                bbt = singles.tile([128, CH], F32)
                nc.scalar.dma_start(out=bbt, in_=bd.to_broadcast((128, CH)))
            if not trivial_b2:
                b2t = singles.tile([128, D], F32)
                nc.scalar.dma_start(out=b2t, in_=b2d.to_broadcast((128, D)))

            for t in range(ntiles):
                rsl = bass.ds(t * 128, 128)
                # ---- load x tile (pre-transposed on host when split) ----
                mk = io.tile([128, 1], F32)
                nc.sync.dma_start(out=mk, in_=mks[rsl, :])
                if split_h:
                    xT = io.tile([128, IN_CH // 128, 128], dt_h)
                    nc.gpsimd.dma_start(
                        out=xT,
                        in_=xs[:, rsl].rearrange("(kc p) r -> p kc r", p=128))
                    xTl = io.tile([128, IN_CH // 128, 128], F32R)
                    nc.gpsimd.dma_start(
                        out=xTl,
                        in_=xl[:, rsl].rearrange("(kc p) r -> p kc r", p=128))
                else:
                    xt = io.tile([128, IN_CH], dt_h)
                    nc.sync.dma_start(out=xt, in_=xs[rsl, :])
                    xT = work.tile([128, IN_CH // 128, 128], dt_h)
                    tx = ptx.tile([128, 512], F32, tag="tx")
                    txv = tx.bitcast(dt_h) if dt_h is F32R else tx
                    for j in range(IN_CH // 128):
                        nc.tensor.transpose(txv[:, bass.ds(j * 128, 128)],
                                            xt[:, bass.ds(j * 128, 128)],
                                            ident_r if dt_h is F32R else ident)
                    nc.vector.tensor_copy(xT.rearrange("p a b -> p (a b)"), txv)

                # ---- h = x @ W1, streamed per 512-quarter ----
                hraw = work.tile([128, CH], F32)
                stats = work.tile([128, 4, 6], F32)
                for nb in range(4):
                    hq = ph.tile([128, 512], F32, tag="hq")
                    nsl = bass.ds(nb * 512, 512)
                    if split_h:
                        nmm = 3 * (IN_CH // 128)
                        i = 0
                        for xop, wop in ((xT, w1sb), (xTl, w1sb), (xT, w1lsb)):
                            for kc in range(IN_CH // 128):
                                nc.tensor.matmul(hq, xop[:, kc, :], wop[:, kc, nsl],
                                                 start=(i == 0), stop=(i == nmm - 1))
                                i += 1
                    else:
                        for kc in range(IN_CH // 128):
                            nc.tensor.matmul(hq, xT[:, kc, :],
                                             w1sb[:, kc, nsl],
                                             start=(kc == 0), stop=(kc == IN_CH // 128 - 1))
                    nc.vector.bn_stats(stats[:, nb, :], hq)
                    nc.scalar.copy(hraw[:, bass.ds(nb * 512, 512)], hq)

                # ---- LayerNorm stats ----
                mv = work.tile([128, 2], F32)
                nc.vector.bn_aggr(mv, stats)
                lnv = work.tile([128, 1], F32)
                nc.scalar.activation(lnv, mv[:, 1:2], AF.Ln, bias=eps_t, scale=1.0)
                rstd = work.tile([128, 1], F32)
                nc.scalar.activation(rstd, lnv, AF.Exp, scale=-0.5)
                nmr = work.tile([128, 1], F32)
                nc.vector.scalar_tensor_tensor(nmr, in0=mv[:, 0:1], scalar=-1.0,
                                               in1=rstd, op0=OP.mult, op1=OP.mult)

                # ---- normalize + (g,b) + relu ----
                nh = work.tile([128, CH], F32)
                if trivial_ln:
                    for nb in range(4):
                        nc.scalar.activation(nh[:, bass.ds(nb * 512, 512)],
                                             hraw[:, bass.ds(nb * 512, 512)],
                                             AF.Relu, bias=nmr, scale=rstd)
                else:
                    t0 = work.tile([128, CH], F32, tag="hraw")
                    nc.vector.tensor_scalar(t0, hraw, rstd, nmr, op0=OP.mult, op1=OP.add)
                    t1 = work.tile([128, CH], F32, tag="hraw")
                    nc.vector.tensor_tensor(t1, t0, gbt, op=OP.mult)
                    t2 = work.tile([128, CH], F32, tag="hraw")
                    nc.vector.tensor_tensor(t2, t1, bbt, op=OP.add)
                    nc.vector.tensor_scalar_max(nh, t2, 0.0)

                # ---- transpose nh: 16x PE transposes in 4 packed banks ----
                hT = work.tile([128, CH // 128, 128], dt_z)
                for g in range(4):
                    th = pth.tile([128, 512], F32, tag="th")
                    for j in range(4):
                        nc.tensor.transpose(th[:, bass.ds(j * 128, 128)],
                                            nh[:, bass.ds((g * 4 + j) * 128, 128)],
                                            ident)
                    dst = hT[:, bass.ds(g * 4, 4), :].rearrange("p a b -> p (a b)")
                    if g % 2 == 0:
                        nc.vector.tensor_copy(dst, th)
                    else:
                        nc.scalar.copy(dst, th)

                # ---- z = nh @ W2 ----
                zq = pz.tile([128, D], F32, tag="zq")
                for kc in range(CH // 128):
                    nc.tensor.matmul(zq, hT[:, kc, :], w2sb[:, kc, :],
                                     start=(kc == 0), stop=(kc == CH // 128 - 1))
                zsb = work.tile([128, D], F32)
                if trivial_b2:
                    nc.scalar.copy(zsb, zq)
                else:
                    nc.vector.tensor_tensor(zsb, zq, b2t, op=OP.add)
                nc.sync.dma_start(out=z_out[rsl, :], in_=zsb)
                if stages < 2:
                    continue

                # ---- softmax pieces (per head, no max-subtraction) ----
                esb = work.tile([128, D], F32)
                nc.scalar.activation(esb, zsb, AF.Exp)
                s4 = work.tile([128, NHEAD], F32)
                nc.vector.reduce_sum(s4, esb.rearrange("p (h d) -> p h d", h=NHEAD),
                                     axis=AX.X)
                logs = work.tile([128, NHEAD], F32)
                nc.scalar.activation(logs, s4, AF.Ln)
                csum = work.tile([128, 1], F32)
                nc.vector.reduce_sum(csum, logs, axis=AX.X)
                rs = work.tile([128, NHEAD], F32)
                nc.vector.reciprocal(rs, s4)
                exsb = work.tile([128, D], F32)
                for h in range(NHEAD):
                    nc.vector.tensor_scalar_mul(exsb[:, bass.ds(h * HD, HD)],
                                                esb[:, bass.ds(h * HD, HD)],
                                                rs[:, h:h + 1])

                # selfz = sum(ex * z); exc = sum(ex * c)
                junk = work.tile([128, D], F32)
                nc.vector.tensor_mul(junk, exsb, zsb)
                selfz = work.tile([128, 1], F32)
                nc.vector.reduce_sum(selfz, junk, axis=AX.X)
                junk2 = work.tile([128, D], F32)
                nc.vector.tensor_mul(junk2, exsb, cb)
                exc = work.tile([128, 1], F32)
                nc.vector.reduce_sum(exc, junk2, axis=AX.X)

                # ---- transpose ex: 2x PE transposes in one bank ----
                exT = work.tile([128, D // 128, 128], dt_div)
                te = ppq.tile([128, 512], F32, tag="pq")
                for j in range(D // 128):
                    nc.tensor.transpose(te[:, bass.ds(j * 128, 128)],
                                        exsb[:, bass.ds(j * 128, 128)], ident)
                if split_div:
                    nc.vector.tensor_copy(exT.rearrange("p a b -> p (a b)"),
                                          te[:, 0:D])
                    exTl = work.tile([128, D // 128, 128], F32R)
                    nc.vector.scalar_tensor_tensor(
                        exTl.rearrange("p a b -> p (a b)"), in0=te[:, 0:D],
                        scalar=1.0, in1=exT.rearrange("p a b -> p (a b)"),
                        op0=OP.mult, op1=OP.subtract)
                else:
                    nc.scalar.copy(exT.rearrange("p a b -> p (a b)"), te[:, 0:D])

                # ---- P = ex @ DeltaT  (argmin div == argmax P) ----
                psb = work.tile([128, M], F32)
                for half in range(2):
                    pq = ppq.tile([128, 512], F32, tag="pq")
                    hsl = bass.ds(half * 512, 512)
                    if split_div:
                        i = 0
                        for lhsv, rhsv in ((exT, dcsb), (exTl, dcsb), (exT, dclsb)):
                            for kc in range(D // 128):
                                nc.tensor.matmul(pq, lhsv[:, kc, :],
                                                 rhsv[:, kc, hsl],
                                                 start=(i == 0), stop=(i == 5))
                                i += 1
                    else:
                        for kc in range(D // 128):
                            nc.tensor.matmul(pq, exT[:, kc, :], dcsb[:, kc, hsl],
                                             start=(kc == 0), stop=(kc == D // 128 - 1))
                    nc.scalar.copy(psb[:, bass.ds(half * 512, 512)], pq)

                # ---- argmax + kl ----
                mx8 = work.tile([128, 8], F32)
                nc.vector.max(mx8, psb)
                ix8 = work.tile([128, 8], mybir.dt.uint16)
                nc.vector.max_index(ix8, mx8, psb)

                # kl = selfz - csum - exc - maxP   (div value at the argmin)
                kl0 = work.tile([128, 1], F32)
                nc.vector.tensor_sub(kl0, selfz, csum)
                kl1 = work.tile([128, 1], F32)
                nc.vector.tensor_sub(kl1, kl0, exc)
                kl2 = work.tile([128, 1], F32)
                nc.vector.tensor_sub(kl2, kl1, mx8[:, 0:1])
                klm = work.tile([128, 1], F32)
                nc.vector.tensor_mul(klm, kl2, mk)
                nc.vector.tensor_add(klacc, klacc, klm)

                if stages < 3:
                    continue
                # ---- index -> DRAM -> wrapped layout -> gather ----
                nc.sync.dma_start(out=idx_scr[rsl, :],
                                  in_=ix8[:, 0:1].bitcast(mybir.dt.int16))
                idxg = work.tile([128, 8], mybir.dt.int16)
                wrapped = idx_scr[rsl, :].rearrange("(f p) one -> p (f one)", p=16)
                for g in range(8):
                    nc.sync.dma_start(out=idxg[g * 16:(g + 1) * 16, :], in_=wrapped)
                qg = work.tile([128, 1, D], F32)
                nc.gpsimd.dma_gather(out_ap=qg, in_ap=emb, idxs_ap=idxg,
                                     num_idxs=128, num_idxs_reg=128, elem_size=D)
                nc.sync.dma_start(out=q_out[rsl, :], in_=qg[:, 0, :])

            # ---- loss partial: sum_p klacc ----
            lps = ppq.tile([1, 1], F32, tag="pq")
            nc.tensor.matmul(lps, klacc, ones_t, start=True, stop=True)
            lsb = work.tile([1, 1], F32)
            nc.vector.tensor_copy(lsb, lps)
            nc.sync.dma_start(out=lsum, in_=lsb)

    nc.compile()
    return nc


LAST_RESULTS = None


def _prepare_host(W1, ln_g, ln_b, W2, b2, embedding):
    trivial_ln = bool(np.all(ln_g == 1.0) and np.all(ln_b == 0.0))
    trivial_b2 = bool(np.all(b2 == 0.0))
    logE = np.log(embedding.astype(np.float64))       # [M, D]
    c = logE.mean(axis=0)                             # [D]
    delta_t = np.ascontiguousarray((logE - c).T.astype(np.float32))  # [D, M]
    cvec = np.ascontiguousarray(c.astype(np.float32)[None, :])       # [1, D]
    return trivial_ln, trivial_b2, delta_t, cvec


def kernel(x, masks, W1, ln_g, ln_b, W2, b2, embedding):
    global LAST_RESULTS
    x = np.ascontiguousarray(np.asarray(x, dtype=np.float32))
    masks = np.ascontiguousarray(np.asarray(masks, dtype=np.float32))
    W1 = np.ascontiguousarray(np.asarray(W1, dtype=np.float32))
    ln_g = np.asarray(ln_g, dtype=np.float32)
    ln_b = np.asarray(ln_b, dtype=np.float32)
    W2 = np.ascontiguousarray(np.asarray(W2, dtype=np.float32))
    b2 = np.asarray(b2, dtype=np.float32)
    embedding = np.ascontiguousarray(np.asarray(embedding, dtype=np.float32))

    trivial_ln, trivial_b2, delta_t, cvec = _prepare_host(
        W1, ln_g, ln_b, W2, b2, embedding)

    nc = build(trivial_ln=trivial_ln, trivial_b2=trivial_b2, split_h=SPLIT_H,
               split_div=SPLIT_DIV)

    xf = x.reshape(-1, IN_CH)
    mf = masks.reshape(-1, 1)
    if SPLIT_H:
        xh_all, xl_all = _split_hi_lo(xf)
        w1h, w1lo = _split_hi_lo(W1)
        xh_all_t = np.ascontiguousarray(xh_all.T)
        xl_all_t = np.ascontiguousarray(xl_all.T)
    if SPLIT_DIV:
        dct_hi, dct_lo = _split_hi_lo(delta_t)
    in_maps = []
    for ci in range(N_CORES):
        sl = slice(ci * ROWS_PER_CORE, (ci + 1) * ROWS_PER_CORE)
        m = {
            "mks": np.ascontiguousarray(mf[sl]),
            "w2": W2,
            "cvec": cvec,
            "emb": embedding,
        }
        if SPLIT_DIV:
            m["dct"] = dct_hi
            m["dctl"] = dct_lo
        else:
            m["dct"] = delta_t
        if SPLIT_H:
            m["xs"] = np.ascontiguousarray(xh_all_t[:, sl])
            m["xl"] = np.ascontiguousarray(xl_all_t[:, sl])
            m["w1"] = w1h
            m["w1l"] = w1lo
        else:
            m["xs"] = np.ascontiguousarray(xf[sl])
            m["w1"] = W1
        if not trivial_ln:
            m["gd"] = np.ascontiguousarray(ln_g[None, :])
            m["bd"] = np.ascontiguousarray(ln_b[None, :])
        if not trivial_b2:
            m["b2d"] = np.ascontiguousarray(b2[None, :])
        in_maps.append(m)

    res = bass_utils.run_bass_kernel_spmd(nc, in_maps, core_ids=list(range(N_CORES)))
    LAST_RESULTS = res

    z = np.concatenate([res.results[c]["z_out"] for c in range(N_CORES)], axis=0)
    q = np.concatenate([res.results[c]["q_out"] for c in range(N_CORES)], axis=0)
    partial = sum(float(res.results[c]["lsum"][0, 0]) for c in range(N_CORES))
    loss = np.float32(COMMIT * partial / B)

    z = z.reshape(B, T, D)
    q = q.reshape(B, T, D)
    return z, q, loss


# revision 37
# speedup vs baseline: 1.0065x; 1.0065x over previous
"""Trainium2 Bass kernel for nn_MultiHeadInfoQuantizer.

Encoder: Linear(512->2048, no bias) -> LayerNorm -> ReLU -> Linear(2048->256)
Per-head (4x64) log_softmax, KL-nearest codebook lookup (1024 codes),
straight-through quantized output, masked commitment loss.

Sharding: data-parallel over N = B*T = 16384 rows; 8 cores x 2048 rows.
Weights/codebook replicated. Scalar loss partial-summed per core and
combined on host.

Returns (z, q_st, loss) matching the reference's structure.
"""

import numpy as np

import concourse.bass as bass
import concourse.tile as tile
from concourse import bacc, mybir
import concourse.bass_utils as bass_utils
from concourse.masks import make_identity

F32 = mybir.dt.float32
F32R = mybir.dt.float32r

B, T = 16, 1024
IN_CH, CH, D, M = 512, 2048, 256, 1024
NHEAD, HD = 4, 64
N_CORES = 8
ROWS_PER_CORE = B * T // N_CORES  # 2048
LN_EPS = 1e-5
COMMIT = 0.25

# Matmul dtype config: "f32" (exact, 4 cyc/row) or "f32r" (~1.6e-4 rel, 1 cyc/row)
DT_H = "f32"
DT_Z = "f32"
DT_DIV = "f32"
# Split-path h matmul: x/W1 decomposed into 11-bit-mantissa hi+lo parts and
# contracted with 3 fp32r matmuls (exact for <=11-bit inputs) -> fp32-quality
# at 3 cyc/row instead of 4.
SPLIT_H = True
SPLIT_DIV = True


def _trunc11(x):
    return (x.view(np.uint32) & np.uint32(0xFFFFF000)).view(np.float32)


def _split_hi_lo(x):
    hi = _trunc11(x)
    lo = _trunc11((x - hi).astype(np.float32))
    return np.ascontiguousarray(hi), np.ascontiguousarray(lo)

AX = mybir.AxisListType
AF = mybir.ActivationFunctionType
OP = mybir.AluOpType

# ---------------------------------------------------------------------------
# Pin the ACT piecewise-poly table choice to the one set that contains every
# function this kernel uses (Exp, Ln, Relu, Copy, Square).  The stock
# insert_act_table_loads pass picks per-function greedily and thrashes
# between exp_and_others / natural_log (~2.7us per switch, twice per tile).
# We only alter which sets the *chooser* believes contain these functions;
# set ids / runtime table contents are untouched.
_COMBINED_SET = "natural_log_exp_and_others"
_orig_get_tables = None


def _patched_get_tables(arch):
    import concourse.hw_specs as hw_specs
    tabs = _orig_get_tables(arch)
    pinned = {AF.Exp, AF.Ln, AF.Relu, AF.Copy, AF.Square, AF.Identity}
    out = {}
    for name, fns in tabs.items():
        if name == _COMBINED_SET:
            out[name] = set(fns)
        else:
            out[name] = set(fns) - pinned
    return out


def _install_table_patch():
    global _orig_get_tables
    if _orig_get_tables is None:
        import concourse.hw_specs as hw_specs
        _orig_get_tables = hw_specs.get_activation_tables
        bacc.get_activation_tables = _patched_get_tables


def _dt(name):
    return F32R if name == "f32r" else F32


def build(ntiles=ROWS_PER_CORE // 128, dt_h=None, dt_z=None, dt_div=None,
          trivial_ln=True, trivial_b2=True, stages=3, split_h=False,
          split_div=False):
    """Build the per-core SPMD program. Each core handles ntiles*128 rows.

    stages: 1 = through z output only; 2 = + softmax/P/argmax/loss; 3 = full.
    """
    _install_table_patch()
    dt_h = _dt(dt_h or DT_H)
    dt_z = _dt(dt_z or DT_Z)
    dt_div = _dt(dt_div or DT_DIV)
    rows = ntiles * 128

    nc = bacc.Bacc("TRN2", target_bir_lowering=False, debug=False,
                   num_devices=N_CORES)

    # ---- DRAM tensors ----
    if split_h:
        xs = nc.dram_tensor("xs", [IN_CH, rows], F32R, kind="ExternalInput").ap()
        xl = nc.dram_tensor("xl", [IN_CH, rows], F32R, kind="ExternalInput").ap()
        w1 = nc.dram_tensor("w1", [IN_CH, CH], F32R, kind="ExternalInput").ap()
        w1l = nc.dram_tensor("w1l", [IN_CH, CH], F32R, kind="ExternalInput").ap()
        dt_h = F32R
    else:
        xs = nc.dram_tensor("xs", [rows, IN_CH], F32, kind="ExternalInput").ap()
        w1 = nc.dram_tensor("w1", [IN_CH, CH], dt_h, kind="ExternalInput").ap()
    mks = nc.dram_tensor("mks", [rows, 1], F32, kind="ExternalInput").ap()
    w2 = nc.dram_tensor("w2", [CH, D], dt_z, kind="ExternalInput").ap()
    if split_div:
        dt_div = F32R
    dct = nc.dram_tensor("dct", [D, M], dt_div, kind="ExternalInput").ap()
    if split_div:
        dctl = nc.dram_tensor("dctl", [D, M], F32R, kind="ExternalInput").ap()
    cvec = nc.dram_tensor("cvec", [1, D], F32, kind="ExternalInput").ap()
    emb = nc.dram_tensor("emb", [M, D], F32, kind="ExternalInput").ap()
    if not trivial_ln:
        gd = nc.dram_tensor("gd", [1, CH], F32, kind="ExternalInput").ap()
        bd = nc.dram_tensor("bd", [1, CH], F32, kind="ExternalInput").ap()
    if not trivial_b2:
        b2d = nc.dram_tensor("b2d", [1, D], F32, kind="ExternalInput").ap()

    z_out = nc.dram_tensor("z_out", [rows, D], F32, kind="ExternalOutput").ap()
    q_out = nc.dram_tensor("q_out", [rows, D], F32, kind="ExternalOutput").ap()
    lsum = nc.dram_tensor("lsum", [1, 1], F32, kind="ExternalOutput").ap()
    idx_scr = nc.dram_tensor("idx_scr", [rows, 1], mybir.dt.int16,
                             kind="Internal").ap()

    with tile.TileContext(nc) as tc:
        with (
            tc.tile_pool(name="singles", bufs=1) as singles,
            tc.tile_pool(name="io", bufs=3) as io,
            tc.tile_pool(name="work", bufs=2) as work,
            tc.tile_pool(name="ph", bufs=4 if split_h else 2, space="PSUM") as ph,
            tc.tile_pool(name="ptx", bufs=2, space="PSUM") as ptx,
            tc.tile_pool(name="pth", bufs=2, space="PSUM") as pth,
            tc.tile_pool(name="ppq", bufs=1, space="PSUM") as ppq,
            tc.tile_pool(name="pz", bufs=1, space="PSUM") as pz,
        ):
            # ---- residents ----
            ident = singles.tile([128, 128], F32)
            make_identity(nc, ident)
            if split_h or dt_h is F32R:
                ident_r = singles.tile([128, 128], F32R)
                nc.vector.tensor_copy(ident_r, ident)
            eps_t = singles.tile([128, 1], F32)
            nc.vector.memset(eps_t, LN_EPS)
            ones_t = singles.tile([128, 1], F32)
            nc.vector.memset(ones_t, 1.0)
            klacc = singles.tile([128, 1], F32)
            nc.vector.memset(klacc, 0.0)

            w1sb = singles.tile([128, IN_CH // 128, CH], dt_h)
            w1r = w1.rearrange("(kc p) n -> p kc n", p=128)
            nc.sync.dma_start(out=w1sb[:, 0, :], in_=w1r[:, 0, :])
            for kc in range(1, IN_CH // 128):
                nc.scalar.dma_start(out=w1sb[:, kc, :], in_=w1r[:, kc, :])
            w2sb = singles.tile([128, CH // 128, D], dt_z)
            if split_h:
                w1lsb = singles.tile([128, IN_CH // 128, CH], F32R)
                w1lr = w1l.rearrange("(kc p) n -> p kc n", p=128)
                for kc in range(IN_CH // 128):
                    nc.scalar.dma_start(out=w1lsb[:, kc, :], in_=w1lr[:, kc, :])
            nc.scalar.dma_start(out=w2sb, in_=w2.rearrange("(kc p) n -> p kc n", p=128))
            dcsb = singles.tile([128, D // 128, M], dt_div)
            nc.scalar.dma_start(out=dcsb, in_=dct.rearrange("(kc p) m -> p kc m", p=128))
            if split_div:
                dclsb = singles.tile([128, D // 128, M], F32R)
                nc.scalar.dma_start(out=dclsb,
                                    in_=dctl.rearrange("(kc p) m -> p kc m", p=128))
            cb = singles.tile([128, D], F32)
            nc.gpsimd.dma_start(out=cb, in_=cvec.to_broadcast((128, D)))
            if not trivial_ln:
                gbt = singles.tile([128, CH], F32)
                nc.scalar.dma_start(out=gbt, in_=gd.to_broadcast((128, CH)))
                bbt = singles.tile([128, CH], F32)
                nc.scalar.dma_start(out=bbt, in_=bd.to_broadcast((128, CH)))
            if not trivial_b2:
                b2t = singles.tile([128, D], F32)
                nc.scalar.dma_start(out=b2t, in_=b2d.to_broadcast((128, D)))

            for t in range(ntiles):
                rsl = bass.ds(t * 128, 128)
                # ---- load x tile (pre-transposed on host when split) ----
                mk = io.tile([128, 1], F32)
                nc.sync.dma_start(out=mk, in_=mks[rsl, :])
                if split_h:
                    xT = io.tile([128, IN_CH // 128, 128], dt_h)
                    nc.gpsimd.dma_start(
                        out=xT,
                        in_=xs[:, rsl].rearrange("(kc p) r -> p kc r", p=128))
                    xTl = io.tile([128, IN_CH // 128, 128], F32R)
                    nc.gpsimd.dma_start(
                        out=xTl,
                        in_=xl[:, rsl].rearrange("(kc p) r -> p kc r", p=128))
                else:
                    xt = io.tile([128, IN_CH], dt_h)
                    nc.sync.dma_start(out=xt, in_=xs[rsl, :])
                    xT = work.tile([128, IN_CH // 128, 128], dt_h)
                    tx = ptx.tile([128, 512], F32, tag="tx")
                    txv = tx.bitcast(dt_h) if dt_h is F32R else tx
                    for j in range(IN_CH // 128):
                        nc.tensor.transpose(txv[:, bass.ds(j * 128, 128)],
                                            xt[:, bass.ds(j * 128, 128)],
                                            ident_r if dt_h is F32R else ident)
                    nc.vector.tensor_copy(xT.rearrange("p a b -> p (a b)"), txv)

                # ---- h = x @ W1, streamed per 512-quarter ----
                hraw = work.tile([128, CH], F32)
                stats = work.tile([128, 4, 6], F32)
                for nb in range(4):
                    hq = ph.tile([128, 512], F32, tag="hq")
                    nsl = bass.ds(nb * 512, 512)
                    if split_h:
                        nmm = 3 * (IN_CH // 128)
                        i = 0
                        for xop, wop in ((xT, w1sb), (xTl, w1sb), (xT, w1lsb)):
                            for kc in range(IN_CH // 128):
                                nc.tensor.matmul(hq, xop[:, kc, :], wop[:, kc, nsl],
                                                 start=(i == 0), stop=(i == nmm - 1))
                                i += 1
                    else:
                        for kc in range(IN_CH // 128):
                            nc.tensor.matmul(hq, xT[:, kc, :],
                                             w1sb[:, kc, nsl],
                                             start=(kc == 0), stop=(kc == IN_CH // 128 - 1))
                    nc.vector.bn_stats(stats[:, nb, :], hq)
                    nc.scalar.copy(hraw[:, bass.ds(nb * 512, 512)], hq)

                # ---- LayerNorm stats ----
                mv = work.tile([128, 2], F32)
                nc.vector.bn_aggr(mv, stats)
                lnv = work.tile([128, 1], F32)
                nc.scalar.activation(lnv, mv[:, 1:2], AF.Ln, bias=eps_t, scale=1.0)
                rstd = work.tile([128, 1], F32)
                nc.scalar.activation(rstd, lnv, AF.Exp, scale=-0.5)
                nmr = work.tile([128, 1], F32)
                nc.vector.scalar_tensor_tensor(nmr, in0=mv[:, 0:1], scalar=-1.0,
                                               in1=rstd, op0=OP.mult, op1=OP.mult)

                # ---- normalize + (g,b) + relu ----
                nh = work.tile([128, CH], F32)
                if trivial_ln:
                    for nb in range(4):
                        nc.scalar.activation(nh[:, bass.ds(nb * 512, 512)],
                                             hraw[:, bass.ds(nb * 512, 512)],
                                             AF.Relu, bias=nmr, scale=rstd)
                else:
                    t0 = work.tile([128, CH], F32, tag="hraw")
                    nc.vector.tensor_scalar(t0, hraw, rstd, nmr, op0=OP.mult, op1=OP.add)
                    t1 = work.tile([128, CH], F32, tag="hraw")
                    nc.vector.tensor_tensor(t1, t0, gbt, op=OP.mult)
                    t2 = work.tile([128, CH], F32, tag="hraw")
                    nc.vector.tensor_tensor(t2, t1, bbt, op=OP.add)
                    nc.vector.tensor_scalar_max(nh, t2, 0.0)

                # ---- transpose nh: 16x PE transposes in 4 packed banks ----
                hT = work.tile([128, CH // 128, 128], dt_z)
                for g in range(4):
                    th = pth.tile([128, 512], F32, tag="th")
                    for j in range(4):
                        nc.tensor.transpose(th[:, bass.ds(j * 128, 128)],
                                            nh[:, bass.ds((g * 4 + j) * 128, 128)],
                                            ident)
                    dst = hT[:, bass.ds(g * 4, 4), :].rearrange("p a b -> p (a b)")
                    if g % 2 == 0:
                        nc.vector.tensor_copy(dst, th)
                    else:
                        nc.scalar.copy(dst, th)

                # ---- z = nh @ W2 ----
                zq = pz.tile([128, D], F32, tag="zq")
                for kc in range(CH // 128):
                    nc.tensor.matmul(zq, hT[:, kc, :], w2sb[:, kc, :],
                                     start=(kc == 0), stop=(kc == CH // 128 - 1))
                zsb = work.tile([128, D], F32)
                if trivial_b2:
                    nc.scalar.copy(zsb, zq)
                else:
                    nc.vector.tensor_tensor(zsb, zq, b2t, op=OP.add)
                nc.sync.dma_start(out=z_out[rsl, :], in_=zsb)
                if stages < 2:
                    continue

                # ---- softmax pieces (per head, no max-subtraction) ----
                esb = work.tile([128, D], F32)
                nc.scalar.activation(esb, zsb, AF.Exp)
                s4 = work.tile([128, NHEAD], F32)
                nc.vector.reduce_sum(s4, esb.rearrange("p (h d) -> p h d", h=NHEAD),
                                     axis=AX.X)
                logs = work.tile([128, NHEAD], F32)
                nc.scalar.activation(logs, s4, AF.Ln)
                csum = work.tile([128, 1], F32)
                nc.vector.reduce_sum(csum, logs, axis=AX.X)
                rs = work.tile([128, NHEAD], F32)
                nc.vector.reciprocal(rs, s4)
                exsb = work.tile([128, D], F32)
                for h in range(NHEAD):
                    nc.vector.tensor_scalar_mul(exsb[:, bass.ds(h * HD, HD)],
                                                esb[:, bass.ds(h * HD, HD)],
                                                rs[:, h:h + 1])

                # selfz = sum(ex * z); exc = sum(ex * c)
                junk = work.tile([128, D], F32)
                nc.vector.tensor_mul(junk, exsb, zsb)
                selfz = work.tile([128, 1], F32)
                nc.vector.reduce_sum(selfz, junk, axis=AX.X)
                junk2 = work.tile([128, D], F32)
                nc.vector.tensor_mul(junk2, exsb, cb)
                exc = work.tile([128, 1], F32)
                nc.vector.reduce_sum(exc, junk2, axis=AX.X)

                # ---- transpose ex: 2x PE transposes in one bank ----
                exT = work.tile([128, D // 128, 128], dt_div)
                te = ppq.tile([128, 512], F32, tag="pq")
                for j in range(D // 128):
                    nc.tensor.transpose(te[:, bass.ds(j * 128, 128)],
                                        exsb[:, bass.ds(j * 128, 128)], ident)
                if split_div:
                    nc.vector.tensor_copy(exT.rearrange("p a b -> p (a b)"),
                                          te[:, 0:D])
                    exTl = work.tile([128, D // 128, 128], F32R)
                    nc.vector.scalar_tensor_tensor(
                        exTl.rearrange("p a b -> p (a b)"), in0=te[:, 0:D],
                        scalar=1.0, in1=exT.rearrange("p a b -> p (a b)"),
                        op0=OP.mult, op1=OP.subtract)
                else:
                    nc.scalar.copy(exT.rearrange("p a b -> p (a b)"), te[:, 0:D])

                # ---- P = ex @ DeltaT  (argmin div == argmax P) ----
                psb = work.tile([128, M], F32)
                for half in range(2):
                    pq = ppq.tile([128, 512], F32, tag="pq")
                    hsl = bass.ds(half * 512, 512)
                    if split_div:
                        i = 0
                        for lhsv, rhsv in ((exT, dcsb), (exTl, dcsb), (exT, dclsb)):
                            for kc in range(D // 128):
                                nc.tensor.matmul(pq, lhsv[:, kc, :],
                                                 rhsv[:, kc, hsl],
                                                 start=(i == 0), stop=(i == 5))
                                i += 1
                    else:
                        for kc in range(D // 128):
                            nc.tensor.matmul(pq, exT[:, kc, :], dcsb[:, kc, hsl],
                                             start=(kc == 0), stop=(kc == D // 128 - 1))
                    nc.scalar.copy(psb[:, bass.ds(half * 512, 512)], pq)

                # ---- argmax + kl ----
                mx8 = work.tile([128, 8], F32)
                nc.vector.max(mx8, psb)
                ix8 = work.tile([128, 8], mybir.dt.uint16)
                nc.vector.max_index(ix8, mx8, psb)

                # kl = selfz - csum - exc - maxP   (div value at the argmin)
                kl0 = work.tile([128, 1], F32)
                nc.vector.tensor_sub(kl0, selfz, csum)
                kl1 = work.tile([128, 1], F32)
                nc.vector.tensor_sub(kl1, kl0, exc)
                kl2 = work.tile([128, 1], F32)
                nc.vector.tensor_sub(kl2, kl1, mx8[:, 0:1])
                klm = work.tile([128, 1], F32)
                nc.vector.tensor_mul(klm, kl2, mk)
                nc.vector.tensor_add(klacc, klacc, klm)

                if stages < 3:
                    continue
                # ---- index -> DRAM -> wrapped layout -> gather ----
                nc.sync.dma_start(out=idx_scr[rsl, :],
                                  in_=ix8[:, 0:1].bitcast(mybir.dt.int16))
                idxg = work.tile([128, 8], mybir.dt.int16)
                wrapped = idx_scr[rsl, :].rearrange("(f p) one -> p (f one)", p=16)
                for g in range(8):
                    nc.sync.dma_start(out=idxg[g * 16:(g + 1) * 16, :], in_=wrapped)
                qg = work.tile([128, 1, D], F32)
                nc.gpsimd.dma_gather(out_ap=qg, in_ap=emb, idxs_ap=idxg,
                                     num_idxs=128, num_idxs_reg=128, elem_size=D)
                nc.sync.dma_start(out=q_out[rsl, :], in_=qg[:, 0, :])

            # ---- loss partial: sum_p klacc ----
            lps = ppq.tile([1, 1], F32, tag="pq")
            nc.tensor.matmul(lps, klacc, ones_t, start=True, stop=True)
            lsb = work.tile([1, 1], F32)
            nc.vector.tensor_copy(lsb, lps)
            nc.sync.dma_start(out=lsum, in_=lsb)

    nc.compile()
    return nc


LAST_RESULTS = None


def _prepare_host(W1, ln_g, ln_b, W2, b2, embedding):
    trivial_ln = bool(np.all(ln_g == 1.0) and np.all(ln_b == 0.0))
    trivial_b2 = bool(np.all(b2 == 0.0))
    logE = np.log(embedding.astype(np.float64))       # [M, D]
    c = logE.mean(axis=0)                             # [D]
    delta_t = np.ascontiguousarray((logE - c).T.astype(np.float32))  # [D, M]
    cvec = np.ascontiguousarray(c.astype(np.float32)[None, :])       # [1, D]
    return trivial_ln, trivial_b2, delta_t, cvec


def kernel(x, masks, W1, ln_g, ln_b, W2, b2, embedding):
    global LAST_RESULTS
    x = np.ascontiguousarray(np.asarray(x, dtype=np.float32))
    masks = np.ascontiguousarray(np.asarray(masks, dtype=np.float32))
    W1 = np.ascontiguousarray(np.asarray(W1, dtype=np.float32))
    ln_g = np.asarray(ln_g, dtype=np.float32)
    ln_b = np.asarray(ln_b, dtype=np.float32)
    W2 = np.ascontiguousarray(np.asarray(W2, dtype=np.float32))
    b2 = np.asarray(b2, dtype=np.float32)
    embedding = np.ascontiguousarray(np.asarray(embedding, dtype=np.float32))

    trivial_ln, trivial_b2, delta_t, cvec = _prepare_host(
        W1, ln_g, ln_b, W2, b2, embedding)

    nc = build(trivial_ln=trivial_ln, trivial_b2=trivial_b2, split_h=SPLIT_H,
               split_div=SPLIT_DIV)

    xf = x.reshape(-1, IN_CH)
    mf = masks.reshape(-1, 1)
    if SPLIT_H:
        xh_all, xl_all = _split_hi_lo(xf)
        w1h, w1lo = _split_hi_lo(W1)
        xh_all_t = np.ascontiguousarray(xh_all.T)
        xl_all_t = np.ascontiguousarray(xl_all.T)
    if SPLIT_DIV:
        dct_hi, dct_lo = _split_hi_lo(delta_t)
    in_maps = []
    for ci in range(N_CORES):
        sl = slice(ci * ROWS_PER_CORE, (ci + 1) * ROWS_PER_CORE)
        m = {
            "mks": np.ascontiguousarray(mf[sl]),
            "w2": W2,
            "cvec": cvec,
            "emb": embedding,
        }
        if SPLIT_DIV:
            m["dct"] = dct_hi
            m["dctl"] = dct_lo
        else:
            m["dct"] = delta_t
        if SPLIT_H:
            m["xs"] = np.ascontiguousarray(xh_all_t[:, sl])
            m["xl"] = np.ascontiguousarray(xl_all_t[:, sl])
            m["w1"] = w1h
            m["w1l"] = w1lo
        else:
            m["xs"] = np.ascontiguousarray(xf[sl])
            m["w1"] = W1
        if not trivial_ln:
            m["gd"] = np.ascontiguousarray(ln_g[None, :])
            m["bd"] = np.ascontiguousarray(ln_b[None, :])
        if not trivial_b2:
            m["b2d"] = np.ascontiguousarray(b2[None, :])
        in_maps.append(m)

    res = bass_utils.run_bass_kernel_spmd(nc, in_maps, core_ids=list(range(N_CORES)))
    LAST_RESULTS = res

    z = np.concatenate([res.results[c]["z_out"] for c in range(N_CORES)], axis=0)
    q = np.concatenate([res.results[c]["q_out"] for c in range(N_CORES)], axis=0)
    partial = sum(float(res.results[c]["lsum"][0, 0]) for c in range(N_CORES))
    loss = np.float32(COMMIT * partial / B)

    z = z.reshape(B, T, D)
    q = q.reshape(B, T, D)
    return z, q, loss


# revision 53
# speedup vs baseline: 1.0191x; 1.0125x over previous
"""Trainium2 Bass kernel for nn_MultiHeadInfoQuantizer.

Encoder: Linear(512->2048, no bias) -> LayerNorm -> ReLU -> Linear(2048->256)
Per-head (4x64) log_softmax, KL-nearest codebook lookup (1024 codes),
straight-through quantized output, masked commitment loss.

Sharding: data-parallel over N = B*T = 16384 rows; 8 cores x 2048 rows.
Weights/codebook replicated. Scalar loss partial-summed per core and
combined on host.

Returns (z, q_st, loss) matching the reference's structure.
"""

import numpy as np

import concourse.bass as bass
import concourse.tile as tile
from concourse import bacc, mybir
import concourse.bass_utils as bass_utils
from concourse.masks import make_identity

F32 = mybir.dt.float32
F32R = mybir.dt.float32r
BF16 = mybir.dt.bfloat16

B, T = 16, 1024
IN_CH, CH, D, M = 512, 2048, 256, 1024
NHEAD, HD = 4, 64
N_CORES = 8
ROWS_PER_CORE = B * T // N_CORES  # 2048
LN_EPS = 1e-5
COMMIT = 0.25

# Matmul dtype config: "f32" (exact, 4 cyc/row) or "f32r" (~1.6e-4 rel, 1 cyc/row)
DT_H = "f32"
DT_Z = "f32"
DT_DIV = "f32"
# Split-path h matmul: x/W1 decomposed into 11-bit-mantissa hi+lo parts and
# contracted with 3 fp32r matmuls (exact for <=11-bit inputs) -> fp32-quality
# at 3 cyc/row instead of 4.
SPLIT_H = True
SPLIT_DIV = True
# z = h@W2 via bf16 hi/lo split: h and W2 each decomposed into bf16 hi+lo
# (~17-bit effective mantissa), transposed on the DMA xbar instead of the PE,
# and contracted with 3 bf16 matmul chains (1 cyc/row) instead of fp32 (4).
import os as _os
Z_BF16 = _os.environ.get("ZBF16", "1") == "1"


def _trunc11(x):
    return (x.view(np.uint32) & np.uint32(0xFFFFF000)).view(np.float32)


def _split_hi_lo(x):
    hi = _trunc11(x)
    lo = _trunc11((x - hi).astype(np.float32))
    return np.ascontiguousarray(hi), np.ascontiguousarray(lo)

AX = mybir.AxisListType
AF = mybir.ActivationFunctionType
OP = mybir.AluOpType

# ---------------------------------------------------------------------------
# Pin the ACT piecewise-poly table choice to the one set that contains every
# function this kernel uses (Exp, Ln, Relu, Copy, Square).  The stock
# insert_act_table_loads pass picks per-function greedily and thrashes
# between exp_and_others / natural_log (~2.7us per switch, twice per tile).
# We only alter which sets the *chooser* believes contain these functions;
# set ids / runtime table contents are untouched.
_COMBINED_SET = "natural_log_exp_and_others"
_orig_get_tables = None


def _patched_get_tables(arch):
    import concourse.hw_specs as hw_specs
    tabs = _orig_get_tables(arch)
    pinned = {AF.Exp, AF.Ln, AF.Relu, AF.Copy, AF.Square, AF.Identity}
    out = {}
    for name, fns in tabs.items():
        if name == _COMBINED_SET:
            out[name] = set(fns)
        else:
            out[name] = set(fns) - pinned
    return out


def _install_table_patch():
    global _orig_get_tables
    if _orig_get_tables is None:
        import concourse.hw_specs as hw_specs
        _orig_get_tables = hw_specs.get_activation_tables
        bacc.get_activation_tables = _patched_get_tables


def _dt(name):
    return F32R if name == "f32r" else F32


def build(ntiles=ROWS_PER_CORE // 128, dt_h=None, dt_z=None, dt_div=None,
          trivial_ln=True, trivial_b2=True, stages=3, split_h=False,
          split_div=False, z_bf16=False):
    """Build the per-core SPMD program. Each core handles ntiles*128 rows.

    stages: 1 = through z output only; 2 = + softmax/P/argmax/loss; 3 = full.
    """
    _install_table_patch()
    dt_h = _dt(dt_h or DT_H)
    dt_z = _dt(dt_z or DT_Z)
    dt_div = _dt(dt_div or DT_DIV)
    rows = ntiles * 128

    nc = bacc.Bacc("TRN2", target_bir_lowering=False, debug=False,
                   num_devices=N_CORES)

    # ---- DRAM tensors ----
    if split_h:
        xs = nc.dram_tensor("xs", [IN_CH, rows], F32R, kind="ExternalInput").ap()
        xl = nc.dram_tensor("xl", [IN_CH, rows], F32R, kind="ExternalInput").ap()
        w1 = nc.dram_tensor("w1", [IN_CH, CH], F32R, kind="ExternalInput").ap()
        w1l = nc.dram_tensor("w1l", [IN_CH, CH], F32R, kind="ExternalInput").ap()
        dt_h = F32R
    else:
        xs = nc.dram_tensor("xs", [rows, IN_CH], F32, kind="ExternalInput").ap()
        w1 = nc.dram_tensor("w1", [IN_CH, CH], dt_h, kind="ExternalInput").ap()
    mks = nc.dram_tensor("mks", [rows, 1], F32, kind="ExternalInput").ap()
    w2 = nc.dram_tensor("w2", [CH, D], dt_z, kind="ExternalInput").ap()
    if split_div:
        dt_div = F32R
    dct = nc.dram_tensor("dct", [D, M], dt_div, kind="ExternalInput").ap()
    if split_div:
        dctl = nc.dram_tensor("dctl", [D, M], F32R, kind="ExternalInput").ap()
    cvec = nc.dram_tensor("cvec", [1, D], F32, kind="ExternalInput").ap()
    emb = nc.dram_tensor("emb", [M, D], F32, kind="ExternalInput").ap()
    if not trivial_ln:
        gd = nc.dram_tensor("gd", [1, CH], F32, kind="ExternalInput").ap()
        bd = nc.dram_tensor("bd", [1, CH], F32, kind="ExternalInput").ap()
    if not trivial_b2:
        b2d = nc.dram_tensor("b2d", [1, D], F32, kind="ExternalInput").ap()

    z_out = nc.dram_tensor("z_out", [rows, D], F32, kind="ExternalOutput").ap()
    q_out = nc.dram_tensor("q_out", [rows, D], F32, kind="ExternalOutput").ap()
    lsum = nc.dram_tensor("lsum", [1, 1], F32, kind="ExternalOutput").ap()
    idx_scr = nc.dram_tensor("idx_scr", [rows, 1], mybir.dt.int16,
                             kind="Internal").ap()

    with tile.TileContext(nc) as tc:
        with (
            tc.tile_pool(name="singles", bufs=1) as singles,
            tc.tile_pool(name="io", bufs=3) as io,
            tc.tile_pool(name="work", bufs=2) as work,
            tc.tile_pool(name="ph", bufs=4 if split_h else 2, space="PSUM") as ph,
            tc.tile_pool(name="ptx", bufs=2, space="PSUM") as ptx,
            tc.tile_pool(name="pth", bufs=2, space="PSUM") as pth,
            tc.tile_pool(name="ppq", bufs=1, space="PSUM") as ppq,
            tc.tile_pool(name="pz", bufs=1, space="PSUM") as pz,
        ):
            # ---- residents ----
            ident = singles.tile([128, 128], F32)
            make_identity(nc, ident)
            if split_h or dt_h is F32R:
                ident_r = singles.tile([128, 128], F32R)
                nc.vector.tensor_copy(ident_r, ident)
            eps_t = singles.tile([128, 1], F32)
            nc.vector.memset(eps_t, LN_EPS)
            ones_t = singles.tile([128, 1], F32)
            nc.vector.memset(ones_t, 1.0)
            klacc = singles.tile([128, 1], F32)
            nc.vector.memset(klacc, 0.0)

            w1sb = singles.tile([128, IN_CH // 128, CH], dt_h)
            w1r = w1.rearrange("(kc p) n -> p kc n", p=128)
            nc.sync.dma_start(out=w1sb[:, 0, :], in_=w1r[:, 0, :])
            for kc in range(1, IN_CH // 128):
                nc.scalar.dma_start(out=w1sb[:, kc, :], in_=w1r[:, kc, :])
            if z_bf16:
                # 2-byte ExternalInputs crash the axon pjrt binding, so the
                # bf16 hi/lo halves of W2 are derived on device from the f32
                # load, staged through a 0.5MB quarter buffer.
                w2hsb = singles.tile([128, CH // 128, D], BF16)
                w2lsb = singles.tile([128, CH // 128, D], BF16)
                w2fq = singles.tile([128, 4, D], F32)
                w2r = w2.rearrange("(kc p) n -> p kc n", p=128)
                for q in range(4):
                    qs = bass.ds(q * 4, 4)
                    nc.scalar.dma_start(out=w2fq, in_=w2r[:, qs, :])
                    nc.scalar.copy(w2hsb[:, qs, :], w2fq)
                    nc.vector.tensor_tensor(w2lsb[:, qs, :], w2fq,
                                            w2hsb[:, qs, :], op=OP.subtract)
            else:
                w2sb = singles.tile([128, CH // 128, D], dt_z)
            if split_h:
                w1lsb = singles.tile([128, IN_CH // 128, CH], F32R)
                w1lr = w1l.rearrange("(kc p) n -> p kc n", p=128)
                for kc in range(IN_CH // 128):
                    nc.scalar.dma_start(out=w1lsb[:, kc, :], in_=w1lr[:, kc, :])
            if not z_bf16:
                nc.scalar.dma_start(out=w2sb,
                                    in_=w2.rearrange("(kc p) n -> p kc n", p=128))
            dcsb = singles.tile([128, D // 128, M], dt_div)
            nc.scalar.dma_start(out=dcsb, in_=dct.rearrange("(kc p) m -> p kc m", p=128))
            if split_div:
                dclsb = singles.tile([128, D // 128, M], F32R)
                nc.scalar.dma_start(out=dclsb,
                                    in_=dctl.rearrange("(kc p) m -> p kc m", p=128))
            cb = singles.tile([128, D], F32)
            nc.gpsimd.dma_start(out=cb, in_=cvec.to_broadcast((128, D)))
            if not trivial_ln:
                gbt = singles.tile([128, CH], F32)
                nc.scalar.dma_start(out=gbt, in_=gd.to_broadcast((128, CH)))
                bbt = singles.tile([128, CH], F32)
                nc.scalar.dma_start(out=bbt, in_=bd.to_broadcast((128, CH)))
            if not trivial_b2:
                b2t = singles.tile([128, D], F32)
                nc.scalar.dma_start(out=b2t, in_=b2d.to_broadcast((128, D)))

            for t in range(ntiles):
                rsl = bass.ds(t * 128, 128)
                # ---- load x tile (pre-transposed on host when split) ----
                mk = io.tile([128, 1], F32)
                nc.sync.dma_start(out=mk, in_=mks[rsl, :])
                if split_h:
                    xT = io.tile([128, IN_CH // 128, 128], dt_h)
                    nc.gpsimd.dma_start(
                        out=xT,
                        in_=xs[:, rsl].rearrange("(kc p) r -> p kc r", p=128))
                    xTl = io.tile([128, IN_CH // 128, 128], F32R)
                    nc.gpsimd.dma_start(
                        out=xTl,
                        in_=xl[:, rsl].rearrange("(kc p) r -> p kc r", p=128))
                else:
                    xt = io.tile([128, IN_CH], dt_h)
                    nc.sync.dma_start(out=xt, in_=xs[rsl, :])
                    xT = work.tile([128, IN_CH // 128, 128], dt_h)
                    tx = ptx.tile([128, 512], F32, tag="tx")
                    txv = tx.bitcast(dt_h) if dt_h is F32R else tx
                    for j in range(IN_CH // 128):
                        nc.tensor.transpose(txv[:, bass.ds(j * 128, 128)],
                                            xt[:, bass.ds(j * 128, 128)],
                                            ident_r if dt_h is F32R else ident)
                    nc.vector.tensor_copy(xT.rearrange("p a b -> p (a b)"), txv)

                # ---- h = x @ W1, streamed per 512-quarter ----
                hraw = work.tile([128, CH], F32)
                stats = work.tile([128, 4, 6], F32)
                for nb in range(4):
                    hq = ph.tile([128, 512], F32, tag="hq")
                    nsl = bass.ds(nb * 512, 512)
                    if split_h:
                        nmm = 3 * (IN_CH // 128)
                        i = 0
                        for xop, wop in ((xT, w1sb), (xTl, w1sb), (xT, w1lsb)):
                            for kc in range(IN_CH // 128):
                                nc.tensor.matmul(hq, xop[:, kc, :], wop[:, kc, nsl],
                                                 start=(i == 0), stop=(i == nmm - 1))
                                i += 1
                    else:
                        for kc in range(IN_CH // 128):
                            nc.tensor.matmul(hq, xT[:, kc, :],
                                             w1sb[:, kc, nsl],
                                             start=(kc == 0), stop=(kc == IN_CH // 128 - 1))
                    nc.vector.bn_stats(stats[:, nb, :], hq)
                    nc.scalar.copy(hraw[:, bass.ds(nb * 512, 512)], hq)

                # ---- LayerNorm stats ----
                mv = work.tile([128, 2], F32)
                nc.vector.bn_aggr(mv, stats)
                lnv = work.tile([128, 1], F32)
                nc.scalar.activation(lnv, mv[:, 1:2], AF.Ln, bias=eps_t, scale=1.0)
                rstd = work.tile([128, 1], F32)
                nc.scalar.activation(rstd, lnv, AF.Exp, scale=-0.5)
                nmr = work.tile([128, 1], F32)
                nc.vector.scalar_tensor_tensor(nmr, in0=mv[:, 0:1], scalar=-1.0,
                                               in1=rstd, op0=OP.mult, op1=OP.mult)

                # ---- normalize + (g,b) + relu ----
                nh = work.tile([128, CH], F32)
                if trivial_ln:
                    for nb in range(4):
                        nc.scalar.activation(nh[:, bass.ds(nb * 512, 512)],
                                             hraw[:, bass.ds(nb * 512, 512)],
                                             AF.Relu, bias=nmr, scale=rstd)
                else:
                    t0 = work.tile([128, CH], F32, tag="hraw")
                    nc.vector.tensor_scalar(t0, hraw, rstd, nmr, op0=OP.mult, op1=OP.add)
                    t1 = work.tile([128, CH], F32, tag="hraw")
                    nc.vector.tensor_tensor(t1, t0, gbt, op=OP.mult)
                    t2 = work.tile([128, CH], F32, tag="hraw")
                    nc.vector.tensor_tensor(t2, t1, bbt, op=OP.add)
                    nc.vector.tensor_scalar_max(nh, t2, 0.0)

                if z_bf16:
                    # ---- split nh into bf16 hi+lo, transpose on the DMA
                    # xbar (PE stays free), contract with 3 bf16 chains ----
                    hhi = work.tile([128, CH], BF16, tag="hhi")
                    nc.scalar.copy(hhi, nh)
                    hlo = work.tile([128, CH], BF16, tag="hlo")
                    nc.vector.tensor_tensor(hlo, nh, hhi, op=OP.subtract)
                    hiT = work.tile([128, CH // 128, 128], BF16, tag="hiT")
                    nc.sync.dma_start_transpose(out=hiT, in_=hhi)
                    loT = work.tile([128, CH // 128, 128], BF16, tag="loT")
                    nc.scalar.dma_start_transpose(out=loT, in_=hlo)
                    zq = pz.tile([128, D], F32, tag="zq")
                    nk = CH // 128
                    for kc in range(nk):
                        nc.tensor.matmul(zq, hiT[:, kc, :], w2hsb[:, kc, :],
                                         start=(kc == 0), stop=False)
                    for kc in range(nk):
                        nc.tensor.matmul(zq, hiT[:, kc, :], w2lsb[:, kc, :],
                                         start=False, stop=False)
                    for kc in range(nk):
                        nc.tensor.matmul(zq, loT[:, kc, :], w2hsb[:, kc, :],
                                         start=False, stop=(kc == nk - 1))
                else:
                    # ---- transpose nh: 16x PE transposes in 4 packed banks ----
                    hT = work.tile([128, CH // 128, 128], dt_z)
                    for g in range(4):
                        th = pth.tile([128, 512], F32, tag="th")
                        for j in range(4):
                            nc.tensor.transpose(th[:, bass.ds(j * 128, 128)],
                                                nh[:, bass.ds((g * 4 + j) * 128, 128)],
                                                ident)
                        dst = hT[:, bass.ds(g * 4, 4), :].rearrange("p a b -> p (a b)")
                        if g % 2 == 0:
                            nc.vector.tensor_copy(dst, th)
                        else:
                            nc.scalar.copy(dst, th)

                    # ---- z = nh @ W2 ----
                    zq = pz.tile([128, D], F32, tag="zq")
                    for kc in range(CH // 128):
                        nc.tensor.matmul(zq, hT[:, kc, :], w2sb[:, kc, :],
                                         start=(kc == 0), stop=(kc == CH // 128 - 1))
                zsb = work.tile([128, D], F32)
                if trivial_b2:
                    nc.scalar.copy(zsb, zq)
                else:
                    nc.vector.tensor_tensor(zsb, zq, b2t, op=OP.add)
                nc.sync.dma_start(out=z_out[rsl, :], in_=zsb)
                if stages < 2:
                    continue

                # ---- softmax pieces (per head, no max-subtraction) ----
                esb = work.tile([128, D], F32)
                nc.scalar.activation(esb, zsb, AF.Exp)
                s4 = work.tile([128, NHEAD], F32)
                nc.vector.reduce_sum(s4, esb.rearrange("p (h d) -> p h d", h=NHEAD),
                                     axis=AX.X)
                logs = work.tile([128, NHEAD], F32)
                nc.scalar.activation(logs, s4, AF.Ln)
                csum = work.tile([128, 1], F32)
                nc.vector.reduce_sum(csum, logs, axis=AX.X)
                rs = work.tile([128, NHEAD], F32)
                nc.vector.reciprocal(rs, s4)
                exsb = work.tile([128, D], F32)
                for h in range(NHEAD):
                    nc.vector.tensor_scalar_mul(exsb[:, bass.ds(h * HD, HD)],
                                                esb[:, bass.ds(h * HD, HD)],
                                                rs[:, h:h + 1])

                # selfz = sum(ex * z); exc = sum(ex * c)
                junk = work.tile([128, D], F32)
                nc.vector.tensor_mul(junk, exsb, zsb)
                selfz = work.tile([128, 1], F32)
                nc.vector.reduce_sum(selfz, junk, axis=AX.X)
                junk2 = work.tile([128, D], F32)
                nc.vector.tensor_mul(junk2, exsb, cb)
                exc = work.tile([128, 1], F32)
                nc.vector.reduce_sum(exc, junk2, axis=AX.X)

                # ---- transpose ex: 2x PE transposes in one bank ----
                exT = work.tile([128, D // 128, 128], dt_div)
                te = ppq.tile([128, 512], F32, tag="pq")
                for j in range(D // 128):
                    nc.tensor.transpose(te[:, bass.ds(j * 128, 128)],
                                        exsb[:, bass.ds(j * 128, 128)], ident)
                if split_div:
                    nc.vector.tensor_copy(exT.rearrange("p a b -> p (a b)"),
                                          te[:, 0:D])
                    exTl = work.tile([128, D // 128, 128], F32R)
                    nc.vector.scalar_tensor_tensor(
                        exTl.rearrange("p a b -> p (a b)"), in0=te[:, 0:D],
                        scalar=1.0, in1=exT.rearrange("p a b -> p (a b)"),
                        op0=OP.mult, op1=OP.subtract)
                else:
                    nc.scalar.copy(exT.rearrange("p a b -> p (a b)"), te[:, 0:D])

                # ---- P = ex @ DeltaT  (argmin div == argmax P) ----
                psb = work.tile([128, M], F32)
                for half in range(2):
                    pq = ppq.tile([128, 512], F32, tag="pq")
                    hsl = bass.ds(half * 512, 512)
                    if split_div:
                        i = 0
                        for lhsv, rhsv in ((exT, dcsb), (exTl, dcsb), (exT, dclsb)):
                            for kc in range(D // 128):
                                nc.tensor.matmul(pq, lhsv[:, kc, :],
                                                 rhsv[:, kc, hsl],
                                                 start=(i == 0), stop=(i == 5))
                                i += 1
                    else:
                        for kc in range(D // 128):
                            nc.tensor.matmul(pq, exT[:, kc, :], dcsb[:, kc, hsl],
                                             start=(kc == 0), stop=(kc == D // 128 - 1))
                    nc.scalar.copy(psb[:, bass.ds(half * 512, 512)], pq)

                # ---- argmax + kl ----
                mx8 = work.tile([128, 8], F32)
                nc.vector.max(mx8, psb)
                ix8 = work.tile([128, 8], mybir.dt.uint16)
                nc.vector.max_index(ix8, mx8, psb)

                # kl = selfz - csum - exc - maxP   (div value at the argmin)
                kl0 = work.tile([128, 1], F32)
                nc.vector.tensor_sub(kl0, selfz, csum)
                kl1 = work.tile([128, 1], F32)
                nc.vector.tensor_sub(kl1, kl0, exc)
                kl2 = work.tile([128, 1], F32)
                nc.vector.tensor_sub(kl2, kl1, mx8[:, 0:1])
                klm = work.tile([128, 1], F32)
                nc.vector.tensor_mul(klm, kl2, mk)
                nc.vector.tensor_add(klacc, klacc, klm)

                if stages < 3:
                    continue
                # ---- index -> DRAM -> wrapped layout -> gather ----
                nc.sync.dma_start(out=idx_scr[rsl, :],
                                  in_=ix8[:, 0:1].bitcast(mybir.dt.int16))
                idxg = work.tile([128, 8], mybir.dt.int16)
                wrapped = idx_scr[rsl, :].rearrange("(f p) one -> p (f one)", p=16)
                for g in range(8):
                    nc.sync.dma_start(out=idxg[g * 16:(g + 1) * 16, :], in_=wrapped)
                qg = work.tile([128, 1, D], F32)
                nc.gpsimd.dma_gather(out_ap=qg, in_ap=emb, idxs_ap=idxg,
                                     num_idxs=128, num_idxs_reg=128, elem_size=D)
                nc.sync.dma_start(out=q_out[rsl, :], in_=qg[:, 0, :])

            # ---- loss partial: sum_p klacc ----
            lps = ppq.tile([1, 1], F32, tag="pq")
            nc.tensor.matmul(lps, klacc, ones_t, start=True, stop=True)
            lsb = work.tile([1, 1], F32)
            nc.vector.tensor_copy(lsb, lps)
            nc.sync.dma_start(out=lsum, in_=lsb)

    nc.compile()
    return nc


LAST_RESULTS = None


def _prepare_host(W1, ln_g, ln_b, W2, b2, embedding):
    trivial_ln = bool(np.all(ln_g == 1.0) and np.all(ln_b == 0.0))
    trivial_b2 = bool(np.all(b2 == 0.0))
    logE = np.log(embedding.astype(np.float64))       # [M, D]
    c = logE.mean(axis=0)                             # [D]
    delta_t = np.ascontiguousarray((logE - c).T.astype(np.float32))  # [D, M]
    cvec = np.ascontiguousarray(c.astype(np.float32)[None, :])       # [1, D]
    return trivial_ln, trivial_b2, delta_t, cvec


def kernel(x, masks, W1, ln_g, ln_b, W2, b2, embedding):
    global LAST_RESULTS
    x = np.ascontiguousarray(np.asarray(x, dtype=np.float32))
    masks = np.ascontiguousarray(np.asarray(masks, dtype=np.float32))
    W1 = np.ascontiguousarray(np.asarray(W1, dtype=np.float32))
    ln_g = np.asarray(ln_g, dtype=np.float32)
    ln_b = np.asarray(ln_b, dtype=np.float32)
    W2 = np.ascontiguousarray(np.asarray(W2, dtype=np.float32))
    b2 = np.asarray(b2, dtype=np.float32)
    embedding = np.ascontiguousarray(np.asarray(embedding, dtype=np.float32))

    trivial_ln, trivial_b2, delta_t, cvec = _prepare_host(
        W1, ln_g, ln_b, W2, b2, embedding)

    nc = build(trivial_ln=trivial_ln, trivial_b2=trivial_b2, split_h=SPLIT_H,
               split_div=SPLIT_DIV, z_bf16=Z_BF16)

    xf = x.reshape(-1, IN_CH)
    mf = masks.reshape(-1, 1)
    if SPLIT_H:
        xh_all, xl_all = _split_hi_lo(xf)
        w1h, w1lo = _split_hi_lo(W1)
        xh_all_t = np.ascontiguousarray(xh_all.T)
        xl_all_t = np.ascontiguousarray(xl_all.T)
    if SPLIT_DIV:
        dct_hi, dct_lo = _split_hi_lo(delta_t)
    in_maps = []
    for ci in range(N_CORES):
        sl = slice(ci * ROWS_PER_CORE, (ci + 1) * ROWS_PER_CORE)
        m = {
            "mks": np.ascontiguousarray(mf[sl]),
            "cvec": cvec,
            "emb": embedding,
        }
        m["w2"] = W2
        if SPLIT_DIV:
            m["dct"] = dct_hi
            m["dctl"] = dct_lo
        else:
            m["dct"] = delta_t
        if SPLIT_H:
            m["xs"] = np.ascontiguousarray(xh_all_t[:, sl])
            m["xl"] = np.ascontiguousarray(xl_all_t[:, sl])
            m["w1"] = w1h
            m["w1l"] = w1lo
        else:
            m["xs"] = np.ascontiguousarray(xf[sl])
            m["w1"] = W1
        if not trivial_ln:
            m["gd"] = np.ascontiguousarray(ln_g[None, :])
            m["bd"] = np.ascontiguousarray(ln_b[None, :])
        if not trivial_b2:
            m["b2d"] = np.ascontiguousarray(b2[None, :])
        in_maps.append(m)

    res = bass_utils.run_bass_kernel_spmd(nc, in_maps, core_ids=list(range(N_CORES)))
    LAST_RESULTS = res

    z = np.concatenate([res.results[c]["z_out"] for c in range(N_CORES)], axis=0)
    q = np.concatenate([res.results[c]["q_out"] for c in range(N_CORES)], axis=0)
    partial = sum(float(res.results[c]["lsum"][0, 0]) for c in range(N_CORES))
    loss = np.float32(COMMIT * partial / B)

    z = z.reshape(B, T, D)
    q = q.reshape(B, T, D)
    return z, q, loss
